# revision 65
# baseline (speedup 1.0000x reference)
"""ContextBranch (context-RoI pooling + 1x1-conv fusion) on 8 Trainium2 cores.

Problem: for each of N=128 boxes, pool the 8 surrounding context cells
(3x3 grid minus center) from a [256, 64, 64] feature map with ROIAlignV2
(7x7 output, sampling_ratio 2), concatenate the 8 pooled chunks into 2048
channels, apply a 1x1 conv (2048->256) + bias + ReLU.

Sharding: box-parallel. Core m handles boxes [16m, 16m+16) and the 128
context cells they consume. The fusion weights are replicated.

Device algorithm (per core), interp-first with a transposed host gather:
  1. ROIAlign collapses to pooled[b,s,:] = sum_p M_b[s,p] * Wnd_b[p,:]
     with M_b = By (x) Bx in [K, 49] and Wnd_b a TIGHT ey x ex feature
     window (K = ex*ey <= 49; the fixed 8x8 window of the baseline wasted
     ~3x DMA on unused rows). Windows/M are computed+gathered on host (the
     walrus build here cannot compile the GPSIMD library reload that the
     on-chip DMAGatherAnt needs).
  2. SPMD slot plan: the 8 cores share one program, so each cell POSITION
     (box-slot n in 0..15, kc in 0..7) gets a window slot sized
     bucket{32,64}(max over cores of that position's K). Slots strip-pack
     into the 128 partitions (32-slots on quarter-rows, 64-slots on
     half-rows); gather+M for a pair of boxes lands in ONE [128, W] DMA.
  3. Interp GEMM per (slot, c_half):
       pooledT[c128, 49] = G^T[Kpix, c128]^T @ M[Kpix, 49]
     as the matmul STATIONARY operand (Ldweights are free). PSUM banks are
     strictly per-(base,size) class: the walrus runtime miscompiles matmul
     sequences that mix input partition ranges within one PSUM bank
     (probed: mixed classes in one bank fail, split banks pass). One
     DVE/Act copy per bank -> SBUF bf16 (GPSIMD cannot access PSUM).
  4. Fusion GEMM per (box, o_half): out[o128, 49] accumulates 16 matmuls
     lhsT=w[(o_hi,kc,c_hi)][c128, o128], rhs=pooledT[c128, 49] in PSUM.
  5. Bias+ReLU fused into the drain: o_hi=0 via scalar-engine ACT bias,
     o_hi=1 via DVE tensor_scalar(add bias, max 0); bf16 staging tiles,
     three merged partition-major out DMAs (pairs 0-3 / 4-6 / 7 per o_hi).
  6. 8 dummy warmup matmuls on memset data pin the PE p-state ramp before
     the first gather chunk lands.
Pairs are processed narrowest-W first (the leading gathers are the PE's
startup critical path); the host unpermutes pair order on reassembly.
The slot plan depends on the boxes; kernel() compiles on first use and
recompiles if the plan changes (cache keyed by the plan).
Host reassembles per-core [128, 8, 2, 2, 49] -> [128, 256, 7, 7].
"""

import numpy as np
import ml_dtypes

import concourse.bass as bass
import concourse.tile as tile
from concourse import mybir
from concourse import bass_utils
from concourse.vector_clock import ScopedClock

# ---------------------------------------------------------------- constants
OUT = 7          # output size
SR = 2           # sampling ratio
SCALE = 1.0 / 16.0
H = W = 64
C = 256
N_BOXES = 128
N_CORES = 8
NB = N_BOXES // N_CORES   # 16 boxes per core
K8 = 8                    # context offsets
NPAIR = NB // 2           # 8 box pairs per core
WIN = 8                   # max per-axis window extent
S49 = OUT * OUT           # 49 pooled positions
SLOTW = C + S49           # 305 cols per slot: [G(256) | M(49)]

BF16 = ml_dtypes.bfloat16

# DMA issue order after pair-0's chunks: "g" = next G pair, "0"-"3" = w
# 1024-col chunk i, "b" = bias. Remaining G pairs append automatically.
DMA_ORDER = ["g", "0", "1", "2", "3", "b"]
# emission order per cycle: True = fuse(np-2) before interp(np)
FUSE_FIRST = False
# drain the last pair's o_hi=1 per b01 quarter
TAIL_SPLIT = False
# issue the mid-kernel out DMAs from the idle GPSIMD queue (SWDGE path) so
# their sem-waits + HWDGE setup don't queue ahead of the final out DMA on SP
OUT_VIA_GPSIMD = True
LAST_O0_VIA_GPSIMD = False
# lag o_hi=1 fusions one extra pipeline cycle behind o_hi=0
STAGGER_O1 = False
# repack+split the first-processed pair at its box boundary
SPLIT_FIRST_PAIR = False


# ------------------------------------------------------- tile drain patch
def _patched_drain_and_barrier(self, tick_clock, wait_clock):
    # The walrus build in this environment rejects >1 sync wait on a Drain
    # ("Too many sync wait commands"), but Tile's kernel-tail drain carries
    # one wait per live semaphore. Split into chained single-wait drains on
    # the same engine, which is semantically identical.
    nc = self.nc
    drain_bi = nc.sync.drain()
    inst = drain_bi.ins
    wait_clock.add_sem_waits(inst, ScopedClock({None: tick_clock.global_clock}))
    si = inst.sync_info
    waits = list(si.on_wait) if si is not None else []
    if len(waits) > 1:
        inst.sync_info = mybir.SyncInfo(on_wait=[waits[0]], on_update=[])
        for w in waits[1:]:
            d2 = nc.sync.drain()
            d2.ins.sync_info = mybir.SyncInfo(on_wait=[w], on_update=[])

    nc.all_engine_barrier()
    assert self.sems is not None
    popped = nc._tile_sem_poison_stack.pop()
    assert popped is self._sem_poison
    nc.clear_and_free_semaphores(list(self.sems.allocated().values()))
    # no trailing barrier: nothing runs after the clears, and the leading
    # barrier already guaranteed every engine consumed its queued waits
    # before any semaphore was rewritten.


tile.TileContext._drain_and_barrier = _patched_drain_and_barrier

_ws_counter = [0]


def _split_multi_waits(nc):
    """Walrus here allows only ONE sync wait per instruction. For every
    instruction carrying N>1 waits, hoist N-1 of them onto injected NoOps on
    the same engine immediately before it (same-engine program order makes
    this semantically identical)."""
    for f in nc.m.functions:
        for blk in f.blocks:
            new_insts = []
            for inst in blk.instructions:
                si = getattr(inst, "sync_info", None)
                waits = list(si.on_wait) if si is not None else []
                if len(waits) > 1:
                    for w in waits[:-1]:
                        _ws_counter[0] += 1
                        nop = mybir.InstNoOp(
                            name=f"I-waitsplit-{_ws_counter[0]}", ins=[], outs=[]
                        )
                        nop.engine = inst.engine
                        nop.sync_info = mybir.SyncInfo(on_wait=[w], on_update=[])
                        nc.register_instruction(nop)
                        new_insts.append(nop)
                    inst.sync_info = mybir.SyncInfo(
                        on_wait=[waits[-1]], on_update=list(si.on_update)
                    )
                new_insts.append(inst)
            blk.instructions = new_insts


# ------------------------------------------------------------- host math
def _context_boxes(boxes):
    """[N,4] -> [8, N, 4] context cells, offset-major (reference order)."""
    boxes = boxes.astype(np.float32)
    x1, y1, x2, y2 = boxes[:, 0], boxes[:, 1], boxes[:, 2], boxes[:, 3]
    w = (x2 - x1) / np.float32(3.0)
    h = (y2 - y1) / np.float32(3.0)
    offs = []
    for i in range(3):
        for j in range(3):
            if i == 1 and j == 1:
                continue
            dx = j * w
            dy = i * h
            offs.append(np.stack([x1 + dx, y1 + dy, x1 + dx + w, y1 + dy + h], axis=1))
    return np.stack(offs, axis=0)


def _axis_weights(lo_c, hi_c, size):
    """Per-axis pooled interp weights for one axis of all B context boxes.

    lo_c, hi_c: [B] box edge coords (image space). Returns (orig [B] int,
    Wax [B, 7, 8] fp32) with pooling (x0.5) folded in.
    """
    B = lo_c.shape[0]
    start = lo_c * np.float32(SCALE) - np.float32(0.5)
    end = hi_c * np.float32(SCALE) - np.float32(0.5)
    bw = (end - start) / np.float32(OUT)
    j = np.arange(OUT * SR)
    t = (j // SR + ((j % SR) + np.float32(0.5)) / np.float32(SR)).astype(np.float32)
    pos = start[:, None] + t[None, :] * bw[:, None]          # [B, 14]
    valid = (pos >= -1.0) & (pos <= size)
    pc = np.clip(pos, np.float32(0.0), np.float32(size - 1))
    lo = np.clip(np.floor(pc), 0.0, size - 2).astype(np.int64)
    f = (pc - lo.astype(np.float32)).astype(np.float32)
    wl = ((1.0 - f) * valid).astype(np.float32)
    wh = (f * valid).astype(np.float32)
    orig = np.clip(lo.min(axis=1), 0, size - WIN)            # [B]
    rel = lo - orig[:, None]                                 # [B, 14] in [0, 6]
    assert rel.min() >= 0 and rel.max() <= WIN - 2
    Wax = np.zeros((B, OUT, WIN), np.float32)
    bi = np.arange(B)
    for jj in range(OUT * SR):
        g = jj // SR
        Wax[bi, g, rel[:, jj]] += 0.5 * wl[:, jj]
        Wax[bi, g, rel[:, jj] + 1] += 0.5 * wh[:, jj]
    return orig, Wax


def _cell_geometry(boxes):
    """Per-cell tight windows: returns (orig_y, orig_x [1024], ey, ex [1024],
    Wy, Wx [1024,7,8], fy, fx [1024]) with (fy,fx) the first used row/col of
    the 8-window and (ey,ex) the used extents."""
    cb = _context_boxes(boxes).reshape(K8 * N_BOXES, 4)
    ox, Wx = _axis_weights(cb[:, 0], cb[:, 2], W)
    oy, Wy = _axis_weights(cb[:, 1], cb[:, 3], H)

    def support(Wax):
        used = (np.abs(Wax) > 0).any(axis=1)                 # [B, 8]
        first = used.argmax(1)
        last = WIN - used[:, ::-1].argmax(1)
        return first, (last - first)

    fx, ex = support(Wx)
    fy, ey = support(Wy)
    return oy, ox, fy, fx, ey, ex, Wy, Wx


def _slot_plan(boxes):
    """SPMD-shared slot plan. Returns a dict:
      sizes[n16, kc8]     slot partition size (32/64), bucket of max-K
      pair p: slots[(bp,kc)] -> (base, colslot); Wp[p] slot-cols per pair
      chunks[p]: list of (base, size, [(bp,kc), ...<=5]) psum groupings
      paircol[p]: starting col (in elements) of pair p in gmsh
    """
    oy, ox, fy, fx, ey, ex, _, _ = _cell_geometry(boxes)
    Kc = (ey * ex).reshape(1, K8 * N_BOXES)
    # position (core m, box-slot n, kc) -> cell id 8*(16m+n)+kc
    m_, n_, kc_ = np.meshgrid(np.arange(N_CORES), np.arange(NB), np.arange(K8),
                              indexing="ij")
    cid = 8 * (NB * m_ + n_) + kc_
    Kmax = Kc[0][cid].max(axis=0)                            # [16, 8]
    assert Kmax.max() <= 64
    sizes = np.where(Kmax <= 32, 32, 64)                     # [16, 8]

    plan = {"sizes": sizes, "slots": [], "Wp": [], "chunks": [], "paircol": []}
    slots_l, Wp_l, chunks_l = [], [], []
    for p in range(NPAIR):
        cells = [(bp, kc) for bp in range(2) for kc in range(K8)]
        sz = {c: int(sizes[2 * p + c[0], c[1]]) for c in cells}
        big = [c for c in cells if sz[c] == 64]
        small = [c for c in cells if sz[c] == 32]
        cur = [0, 0, 0, 0]                                   # quarter cursors
        slot = {}
        chunks = []
        for c in big:
            h = 0 if cur[0] + cur[1] <= cur[2] + cur[3] else 1
            col = max(cur[2 * h], cur[2 * h + 1])
            slot[c] = (64 * h, col)
            cur[2 * h] = cur[2 * h + 1] = col + 1
        for c in small:
            q = int(np.argmin(cur))
            slot[c] = (32 * q, cur[q])
            cur[q] += 1
        # psum chunks: group by (base, size), <=5 slots per bank
        groups = {}
        for c in cells:
            groups.setdefault((slot[c][0], sz[c]), []).append(c)
        for (base, size), cs in sorted(groups.items()):
            for i in range(0, len(cs), 5):
                chunks.append((base, size, cs[i:i + 5]))
        slots_l.append(slot)
        Wp_l.append(max(cur))
        chunks_l.append(chunks)
    # processing order: narrowest pairs first (the leading gathers are the
    # PE's startup critical path, so smaller first transfers start it
    # earlier); stable sort keeps the rest in original order.
    order = sorted(range(NPAIR), key=lambda p: Wp_l[p])
    if SPLIT_FIRST_PAIR:
        # repack the FIRST-processed pair with box-0's slots in their own
        # leading columns: its gather splits at the box boundary so interp
        # and fusion of box 0 start while box 1 is still on the wire.
        fp = order[0]
        cur = [0, 0, 0, 0]
        slot = {}
        chunks = []
        for bp in range(2):
            if bp == 1:
                cur = [max(cur)] * 4
                plan["p0cut"] = cur[0]
            cells = [(bp, kc) for kc in range(K8)]
            sz = {c: int(sizes[2 * fp + c[0], c[1]]) for c in cells}
            big = [c for c in cells if sz[c] == 64]
            small = [c for c in cells if sz[c] == 32]
            for c in big:
                h = 0 if cur[0] + cur[1] <= cur[2] + cur[3] else 1
                col = max(cur[2 * h], cur[2 * h + 1])
                slot[c] = (64 * h, col)
                cur[2 * h] = cur[2 * h + 1] = col + 1
            for c in small:
                q = int(np.argmin(cur))
                slot[c] = (32 * q, cur[q])
                cur[q] += 1
            groups = {}
            for c in cells:
                groups.setdefault((slot[c][0], sz[c]), []).append(c)
            for (base, size), cs in sorted(groups.items()):
                for i in range(0, len(cs), 5):
                    chunks.append((base, size, cs[i:i + 5]))
        slots_l[fp] = slot
        Wp_l[fp] = max(cur)
        chunks_l[fp] = chunks
    col_off = 0
    for i, op in enumerate(order):
        plan["slots"].append(slots_l[op])
        plan["Wp"].append(Wp_l[op])
        plan["chunks"].append(chunks_l[op])
        plan["paircol"].append(col_off)
        col_off += Wp_l[op] * SLOTW
    plan["order"] = order
    plan["total_cols"] = col_off
    plan["key"] = (tuple(sizes.ravel().tolist()), tuple(plan["Wp"]),
                   tuple(order), plan.get("p0cut", -1))
    return plan


def _prep(features, boxes, w_fuse, b_fuse, plan):
    """All host-side layout/index prep. Returns (shared dict, per-core list)."""
    features = np.asarray(features, np.float32)
    boxes = np.asarray(boxes, np.float32)
    w_fuse = np.asarray(w_fuse, np.float32)
    b_fuse = np.asarray(b_fuse, np.float32)

    oy, ox, fy, fx, ey, ex, Wy, Wx = _cell_geometry(boxes)
    featT = np.ascontiguousarray(features.reshape(C, H * W).T)  # [4096, 256]

    # o_hi-major so fusion's o_hi=0 quads only need the first half of w:
    # w_sb[c_lo, (o_hi, kc, c_hi, o_lo)]
    w5 = w_fuse.reshape(2, 128, K8, 2, 128)
    wsb = np.ascontiguousarray(
        w5.transpose(4, 0, 2, 3, 1).reshape(128, K8 * 2 * 2 * 128)
    ).astype(BF16)
    b2 = np.ascontiguousarray(b_fuse.reshape(2, 128).T).astype(np.float32)
    shared = {"wsb": wsb, "b2": b2}

    per_core = []
    for m in range(N_CORES):
        gm = np.zeros((128, plan["total_cols"]), np.float32)
        for p in range(NPAIR):
            base_col = plan["paircol"][p]
            for (bp, kc), (pbase, cslot) in plan["slots"][p].items():
                n = NB * m + 2 * plan["order"][p] + bp
                b = 8 * n + kc                               # cbox id
                eyc, exc = int(ey[b]), int(ex[b])
                y0, x0 = int(oy[b] + fy[b]), int(ox[b] + fx[b])
                col = base_col + cslot * SLOTW
                # G rows: (iy, ix) y-major over the tight window
                rows = ((y0 + np.arange(eyc))[:, None] * W
                        + (x0 + np.arange(exc))[None, :]).ravel()
                gm[pbase:pbase + eyc * exc, col:col + C] = featT[rows]
                # M rows: Wy[py, fy+iy] * Wx[px, fx+ix], same flattening
                Mc = (Wy[b][:, fy[b]:fy[b] + eyc][:, None, :, None]
                      * Wx[b][:, fx[b]:fx[b] + exc][None, :, None, :])
                Mc = Mc.transpose(2, 3, 0, 1).reshape(eyc * exc, S49)
                gm[pbase:pbase + eyc * exc, col + C:col + SLOTW] = Mc
        per_core.append({"gmsh": np.ascontiguousarray(gm.astype(BF16))})
    return shared, per_core


# ------------------------------------------------------------ device build
def _build_nc(plan):
    nc = bass.Bass("TRN2", target_bir_lowering=False, debug=False,
                   num_devices=N_CORES, dynamic_dma_scratch_size=32768)
    dt = mybir.dt
    wsb = nc.dram_tensor("wsb", [128, K8 * 2 * 2 * 128], dt.bfloat16, kind="ExternalInput").ap()
    b2 = nc.dram_tensor("b2", [128, 2], dt.float32, kind="ExternalInput").ap()
    gmsh = nc.dram_tensor("gmsh", [128, plan["total_cols"]], dt.bfloat16, kind="ExternalInput").ap()
    # partition-major output: [o_lo, np, o_hi, b01, s]
    out = nc.dram_tensor("out", [128, NPAIR, 2, 2, S49], dt.bfloat16, kind="ExternalOutput").ap()

    with tile.TileContext(nc) as tc:
        with (
            tc.tile_pool(name="const", bufs=1) as const,
            tc.tile_pool(name="g", bufs=NPAIR) as gpool,
            tc.tile_pool(name="psb", bufs=22) as psb_pool,
            tc.tile_pool(name="pps", bufs=6, space="PSUM") as pps_pool,
            tc.tile_pool(name="ops", bufs=2, space="PSUM") as ops_pool,
            tc.tile_pool(name="osb", bufs=3) as osb_pool,
        ):
            # --- PE p-state warmup (cost-model ramp); numerically inert.
            warm_sb = const.tile([128, 128 + S49], dt.bfloat16)
            nc.vector.memset(warm_sb[:], 0.0)
            warm_ps = pps_pool.tile([128, 5, 2, S49], dt.float32, tag="ppq")
            for wi in range(8):
                nc.tensor.matmul(
                    warm_ps[:, wi % 5, wi // 5, 0:8], lhsT=warm_sb[:, 0:128],
                    rhs=warm_sb[:, 128:136], start=True, stop=True,
                )

            g_tiles = []
            for np_ in range(NPAIR):
                g_sb = gpool.tile([128, plan["Wp"][np_] * SLOTW], dt.bfloat16,
                                  tag="gt", name=f"g{np_}")
                g_tiles.append(g_sb)
            w_sb = const.tile([128, K8 * 2 * 2 * 128], dt.bfloat16)
            b_sb = const.tile([128, 2], dt.float32)
            st_a = osb_pool.tile([128, 4, 2, 2, S49], dt.bfloat16)   # pairs 0-3
            st_b = osb_pool.tile([128, 4, 2, 2, S49], dt.bfloat16)   # pairs 4-7
            stages = [(st_a, 0), (st_a, 1), (st_a, 2), (st_a, 3),
                      (st_b, 0), (st_b, 1), (st_b, 2), (st_b, 3)]

            # DMA issue order: pair 0 in three slot-aligned chunks (first
            # interp starts ~3.5us), w o_hi=0 half, pair 1, w o_hi=1 half,
            # bias, then pairs 2-7 one DMA each.
            def gdma(np_, c0, c1):
                nc.sync.dma_start(
                    g_tiles[np_][:, c0 * SLOTW:c1 * SLOTW],
                    gmsh[:, plan["paircol"][np_] + c0 * SLOTW:
                         plan["paircol"][np_] + c1 * SLOTW])

            def wdma(i):
                nc.sync.dma_start(w_sb[:, 1024 * i:1024 * (i + 1)],
                                  wsb[:, 1024 * i:1024 * (i + 1)])

            W0 = plan["Wp"][0]
            cuts = ([plan["p0cut"], W0] if "p0cut" in plan else [W0])
            prev = 0
            for cut in cuts:
                gdma(0, prev, cut)
                prev = cut
            order = DMA_ORDER
            emitted_g = 1
            for tok in order:
                if tok == "g":
                    if emitted_g < NPAIR:
                        gdma(emitted_g, 0, plan["Wp"][emitted_g])
                        emitted_g += 1
                elif tok == "b":
                    nc.sync.dma_start(b_sb[:], b2[:])
                else:
                    wdma(int(tok))
            while emitted_g < NPAIR:
                gdma(emitted_g, 0, plan["Wp"][emitted_g])
                emitted_g += 1

            def interp(np_):
                """Interp GEMMs + per-bank PSUM->SBUF copies for pair np_.
                Returns {(bp, kc): (p_sb_tile, idx)}. One (base,size) class
                per PSUM bank (walrus partition-range-mix bug)."""
                g_sb = g_tiles[np_]
                slotmap = {}
                eng_i = 0
                for base, size, cs in plan["chunks"][np_]:
                    ncell = len(cs)
                    p_ps = pps_pool.tile([128, 5, 2, S49], dt.float32,
                                         tag="ppq", name=f"pp{np_}_{base}_{size}")
                    for i, c in enumerate(cs):
                        cslot = plan["slots"][np_][c][1]
                        col = cslot * SLOTW
                        for c_hi in range(2):
                            nc.tensor.matmul(
                                p_ps[:, i, c_hi, :],
                                lhsT=g_sb[base:base + size,
                                          col + c_hi * 128:col + (c_hi + 1) * 128],
                                rhs=g_sb[base:base + size, col + C:col + SLOTW],
                                start=True, stop=True,
                                tile_position=(base, 0),
                            )
                    p_sb = psb_pool.tile([128, 5, 2, S49], dt.bfloat16,
                                         tag="psb", name=f"ps{np_}_{base}_{size}")
                    eng = [nc.vector, nc.scalar][eng_i % 2]
                    src = p_ps[:, 0:ncell, :, :].rearrange("p a b c -> p (a b c)")
                    dst = p_sb[:, 0:ncell, :, :].rearrange("p a b c -> p (a b c)")
                    if eng is nc.scalar:
                        eng.activation(dst, src, mybir.ActivationFunctionType.Copy)
                    else:
                        eng.tensor_copy(dst, src)
                    for i, c in enumerate(cs):
                        slotmap[c] = (p_sb, i, eng_i)
                    eng_i += 1
                return slotmap

            def fuse_half(np_, o_hi, slotmap):
                """Fusion GEMMs for one o_hi (quadrants b01=0,1) + fused
                bias+ReLU (o0: ACT bias; o1: DVE add+max) into staging."""
                last = np_ == NPAIR - 1
                st, srow = stages[np_]
                # separate PSUM tile per o_hi quadrant-pair: sharing one tile
                # creates a WAR edge (o1 matmuls wait for ReLU(o0)'s read)
                o_ps = ops_pool.tile([128, 2, S49], dt.float32,
                                     tag="ops", name=f"op{np_}_{o_hi}")
                def drain(lo, hi):
                    dst = st[:, srow, o_hi, lo:hi, :].rearrange("p a b -> p (a b)")
                    src = o_ps[:, lo:hi, :].rearrange("p a b -> p (a b)")
                    if o_hi == 0:
                        nc.scalar.activation(
                            dst, src, mybir.ActivationFunctionType.Relu,
                            bias=b_sb[:, 0:1],
                        )
                    else:
                        nc.vector.tensor_scalar(
                            dst, src, b_sb[:, 1:2], 0.0,
                            mybir.AluOpType.add, mybir.AluOpType.max,
                        )
                    if last:
                        nc.sync.dma_start(
                            out[:, np_, o_hi, lo:hi, :].rearrange(
                                "p a b -> p (a b)"),
                            st[:, srow, o_hi, lo:hi, :].rearrange("p a b -> p (a b)"))

                # the last pair's o_hi=1 drains per b01 quarter right after
                # that quarter's matmuls, so the final ReLU+DMA chain after
                # the very last matmul is as short as possible.
                split = last and o_hi == 1 and TAIL_SPLIT
                for b01 in range(2):
                    # consume kc in interp-bank completion order so the
                    # quadrant's first matmuls gate on the pair's FIRST
                    # pooled copy, not its last (accumulation commutes).
                    order = sorted(range(K8), key=lambda kc: slotmap[(b01, kc)][2])
                    for j, kc in enumerate(order):
                        p_sb, idx, _ = slotmap[(b01, kc)]
                        for c_hi in range(2):
                            nc.tensor.matmul(
                                o_ps[:, b01, :],
                                lhsT=w_sb[:, ((o_hi * K8 + kc) * 2 + c_hi) * 128:
                                          ((o_hi * K8 + kc) * 2 + c_hi + 1) * 128],
                                rhs=p_sb[:, idx, c_hi, :],
                                start=(j == 0 and c_hi == 0),
                                stop=(j == K8 - 1 and c_hi == 1),
                            )
                    if split:
                        drain(b01, b01 + 1)
                if not split:
                    drain(0, 2)

            # Software pipeline, depth 2 for o_hi=0 fusions and depth 3 for
            # o_hi=1 (STAGGER_O1): the o1 quads need the late-arriving
            # second half of w, so lagging them one more cycle keeps them
            # off the early critical path. The Tile scheduler refines the
            # final order; emission priority still steers it.
            def dma_sta():
                eng = nc.gpsimd if OUT_VIA_GPSIMD else nc.sync
                eng.dma_start(
                    out[:, 0:4, :, :, :].rearrange("p a b c d -> p (a b c d)"),
                    st_a[:].rearrange("p a b c d -> p (a b c d)"))

            def dma_stb():
                eng = nc.gpsimd if OUT_VIA_GPSIMD else nc.sync
                eng.dma_start(
                    out[:, 4:7, :, :, :].rearrange("p a b c d -> p (a b c d)"),
                    st_b[:, 0:3, :, :, :].rearrange("p a b c d -> p (a b c d)"))

            pending = {}
            pending[0] = interp(0)
            pending[1] = interp(1)
            if STAGGER_O1:
                for np_ in range(2, NPAIR):
                    pending[np_] = interp(np_)
                    fuse_half(np_ - 2, 0, pending[np_ - 2])
                    if np_ >= 3:
                        fuse_half(np_ - 3, 1, pending.pop(np_ - 3))
                        if np_ - 3 == 3:
                            dma_sta()
                fuse_half(NPAIR - 2, 0, pending[NPAIR - 2])
                fuse_half(NPAIR - 3, 1, pending.pop(NPAIR - 3))
                fuse_half(NPAIR - 1, 0, pending[NPAIR - 1])
                fuse_half(NPAIR - 2, 1, pending.pop(NPAIR - 2))
                dma_stb()
                fuse_half(NPAIR - 1, 1, pending.pop(NPAIR - 1))
            else:
                for np_ in range(2, NPAIR):
                    pending[np_] = interp(np_)
                    fuse_half(np_ - 2, 0, pending[np_ - 2])
                    fuse_half(np_ - 2, 1, pending.pop(np_ - 2))
                    if np_ - 2 == 3:
                        dma_sta()
                for np_ in (NPAIR - 2, NPAIR - 1):
                    fuse_half(np_, 0, pending[np_])
                    fuse_half(np_, 1, pending.pop(np_))
                    if np_ == NPAIR - 2:
                        dma_stb()
    _split_multi_waits(nc)
    return nc


_NC_CACHE = None
_NC_KEY = None


def _get_nc(plan):
    global _NC_CACHE, _NC_KEY
    if _NC_CACHE is None or _NC_KEY != plan["key"]:
        _NC_CACHE = _build_nc(plan)
        _NC_KEY = plan["key"]
    return _NC_CACHE


def kernel(features, boxes, w_fuse, b_fuse):
    plan = _slot_plan(np.asarray(boxes, np.float32))
    shared, per_core = _prep(features, boxes, w_fuse, b_fuse, plan)
    in_maps = [{**shared, **pc} for pc in per_core]
    nc = _get_nc(plan)
    res = bass_utils.run_bass_kernel_spmd(
        nc, in_maps, core_ids=list(range(N_CORES)), trace=False
    )
    inv = np.argsort(plan["order"])               # processed pos of orig pair
    parts = []
    for m in range(N_CORES):
        o = res.results[m]["out"]                 # [128, proc, 2, 2, 49] bf16
        # unpermute pairs, then [o_lo, np, o_hi, b01, s] -> [np, b01, o_hi, o_lo, s]
        o = np.asarray(o, np.float32)[:, inv]
        o = o.transpose(1, 3, 2, 0, 4).reshape(NB, C, S49)
        parts.append(o)
    full = np.concatenate(parts, axis=0)          # [128, 256, 49]
    out = full.reshape(N_BOXES, C, OUT, OUT)
    return np.ascontiguousarray(out)
